# revision 30
# baseline (speedup 1.0000x reference)
"""Trainium2 Bass kernel for nn_DiscriminativeAlignmentLoss.

loss = 0.5*(CE_row + CE_col) over logits = -dist/T,
dist = (1/sqrt(c)) * arccosh(c*(v_time*t_time - v.t))   (Lorentz pairwise)

Strategy (8 cores, data parallel over v rows), v3 "normalized poly" scheme:
  - Host normalizes both sides by their Lorentz time components:
    v' = 32*v/vt_i, t' = 32*t/tt_j (fp8).  PSUM then holds P = 1024*d'
    with d' = (v.t)/(vt_i*tt_j), |d'| <~ 0.2, and the Lorentz arg factors:
      logit = A_i + B_j - k*ln(1-d'),  A_i = -k*(ln2+ln c+ln vt_i),
                                       B_j = -k*ln tt_j.
    This kills the baseline's rank-4 "time product" tail matmul: PE does
    exactly 3 fp8 DoubleRow matmuls (K=768) per 512-col group (82us/core).
  - A single CUSTOM DVE op (6 ALU stages, 1 pass) computes the whole
    exp argument from PSUM:  x = Horner_cubic(P) + lnW_j
    where the cubic is the minimax fit of -k*ln(1-d') over the observed
    d' range and lnW_j = B_j - maxB streams in as Src1 (fp16, replicated
    across partitions).  This replaces BOTH the baseline's ACT Ln pass
    (the kernel was ScalarE-bound at 80%) and any separate per-column
    weighting pass.
  - ACT does ONE Exp per chunk: et = exp(x + (A_i - SA)) = full shifted
    exp matrix; its accum_out gives row partial sums for free.
  - Column sums are plain fp16 adds (DVE 2x / GPSIMD), m==0 chunks write
    the accumulator directly from the Exp.  All shift/log arithmetic and
    the 128-partition reduction happen on host in fp64.
"""

import numpy as np
import ml_dtypes

import concourse.bass as bass  # noqa: F401  (registers AP machinery)
import concourse.tile as tile
from concourse import bacc, mybir
from concourse import hw_specs as _hw_specs
from concourse.bass_utils import run_bass_kernel_spmd
import concourse.dve_ops as _dve_ops
from concourse.dve_ops import DveOp as _DveOp
from concourse.dve_spec import Spec as _Spec, Src0, Src1, C0, C1, C2
from concourse.dve_spec import lower as _dve_lower
from concourse.dve_uop import DveOpSpec as _DveOpSpec

# The act-table insertion pass resolves each activation to the FIRST set
# containing its function. Keep Exp/Ln pinned to the combined set so a
# single ACT_TABLE_LOAD serves the whole kernel.
_orig_get_activation_tables = _hw_specs.get_activation_tables


def _patched_get_activation_tables(arch):
    tables = _orig_get_activation_tables(arch)
    drop = {mybir.ActivationFunctionType.Ln, mybir.ActivationFunctionType.Exp}
    return {
        name: (funcs if name == "natural_log_exp_and_others" else funcs - drop)
        for name, funcs in tables.items()
    }


bacc.get_activation_tables = _patched_get_activation_tables


# --- custom DVE op: x = ((P*c3 + c2)*P + c1)*P + lnW  (Horner, 6 stages) ---
def _poly3w_ref(in0, in1, s0, s1, imm2):
    x = in0.astype(np.float32)
    return ((x * imm2 + s1) * x + s0) * x + in1


def _register_poly3w():
    if "POLY3W_ANT" in _dve_ops._SUB_OPCODE_FOR_NAME:
        for op in _dve_ops.OPS:
            if op.name == "POLY3W_ANT":
                return op
    spec = _Spec(
        body=((Src0 * C2 + C1) * Src0 + C0) * Src0 + Src1,
        reference=_poly3w_ref,
    )
    shas = {
        v: _DveOpSpec(
            name="POLY3W_ANT", opcode=0, uops=_dve_lower(spec, ver=v), rd1_en=True
        ).sha(v)
        for v in ("v3", "v4")
    }
    op = _DveOp("POLY3W_ANT", spec, subdim=False, uops_sha=shas)
    _dve_ops.OPS.append(op)
    _dve_ops._SUB_OPCODE_FOR_NAME[op.name] = (
        _dve_ops._CUSTOM_DVE_ROW_BASE + len(_dve_ops.OPS) - 1
    )
    return op


_POLY3W = _register_poly3w()

N = 8192
D = 768
NCORES = 8
R = N // NCORES  # 1024 rows per core
MT = 8  # 128-row m-tiles per core
NQ = 4  # 2048-column chunks
KT = 6  # 128-row K subtiles (768 = 6*128)
TEMPERATURE = 0.07
EPS = 1e-6
LN2 = float(np.log(2.0))
FSCALE = 32.0  # fp8 pre-scale of the normalized operands (power of 2)
PSC = FSCALE * FSCALE  # PSUM = PSC * d'
bf16 = ml_dtypes.bfloat16
fp8 = ml_dtypes.float8_e4m3
fp16 = np.float16
dt = mybir.dt

# colacc adds: m-tiles whose add runs on GPSIMD (rest on DVE at 2x fp16).
# m==0 writes direct from the Exp, m==7 stays on DVE to keep the strip-end
# DMA chain short.
GP_ADD_MS = (1, 3, 5)

_program_cache = {}


def _cubic_fit(Rfit):
    """Minimax cubic fit of -ln(1-x) ~ c0+c1 x+c2 x^2+c3 x^3 on [-R, R]."""
    xs = np.cos(np.pi * (np.arange(4000) + 0.5) / 4000) * Rfit
    coef = np.polynomial.chebyshev.chebfit(xs, -np.log1p(-xs), 3)
    return [float(z) for z in np.polynomial.chebyshev.cheb2poly(coef)]


def _build_program(c: float, Rfit: float):
    """Build + compile the per-core Bass program (same on all 8 cores)."""
    k_eff = (1.0 / c) ** 0.5 / TEMPERATURE
    c0, c1, c2, c3 = _cubic_fit(Rfit)
    a1 = k_eff * c1 / PSC
    a2 = k_eff * c2 / (PSC * PSC)
    a3 = k_eff * c3 / (PSC * PSC * PSC)

    nc = bacc.Bacc(
        "TRN2",
        target_bir_lowering=False,
        debug=False,
        enable_asserts=False,
        num_devices=NCORES,
    )

    vt8_d = nc.dram_tensor("vt8", [128, KT, R], dt.float8e4, kind="ExternalInput")
    tt8_d = nc.dram_tensor(
        "tt8", [NQ, 128, KT, 2048], dt.float8e4, kind="ExternalInput"
    )
    wln_d = nc.dram_tensor("wln", [NQ, 128, 2048], dt.float16, kind="ExternalInput")
    bias_d = nc.dram_tensor("bias", [128, MT], dt.float32, kind="ExternalInput")
    rowparts_d = nc.dram_tensor(
        "rowparts", [128, MT * NQ + 12], dt.float32, kind="ExternalOutput"
    )
    colsum_d = nc.dram_tensor("colsum", [128, N], dt.float16, kind="ExternalOutput")

    DR = mybir.MatmulPerfMode.DoubleRow

    with tile.TileContext(nc) as tc:
        with (
            tc.tile_pool(name="consts", bufs=1) as consts,
            tc.tile_pool(name="xpool", bufs=3) as xpool,
            tc.tile_pool(name="epool", bufs=3) as epool,
            tc.tile_pool(name="mmps", bufs=2, space="PSUM") as mmps,
        ):
            tt8_t = [
                consts.tile([128, KT, 2048], dt.float8e4, name=f"tt8_{s}")
                for s in range(NQ)
            ]
            wln_t = [
                consts.tile([128, 2048], dt.float16, name=f"wln_{s}")
                for s in range(NQ)
            ]
            vt8_t = consts.tile([128, KT, R], dt.float8e4, name="vt8_t")
            bias_t = consts.tile([128, MT], dt.float32, name="bias_t")
            rowparts_t = consts.tile(
                [128, MT * NQ + 12], dt.float32, name="rowparts_t"
            )
            colacc = consts.tile([128, N], dt.float16, name="colacc")

            # Input DMAs ride both hardware DGE rings (sync + scalar), in
            # need order at fine grain so the first fill chunks (m0-m3,
            # cols 0:1024 of strip 0) can start ~5us earlier than a bulk
            # strip load would allow: any delay to the first real matmuls
            # lets the HAM clock gate re-throttle the PE after the warmup
            # stream ends. Later strips are split so each arrives well
            # before its chunks run.
            nc.sync.dma_start(out=vt8_t[:, :3, 0:512], in_=vt8_d[:, :3, 0:512])
            nc.scalar.dma_start(out=vt8_t[:, 3:, 0:512], in_=vt8_d[:, 3:, 0:512])
            nc.sync.dma_start(
                out=tt8_t[0][:, :3, 0:1024], in_=tt8_d[0, :, :3, 0:1024]
            )
            nc.scalar.dma_start(
                out=tt8_t[0][:, 3:, 0:1024], in_=tt8_d[0, :, 3:, 0:1024]
            )
            nc.scalar.dma_start(out=wln_t[0][:, 0:1024], in_=wln_d[0, :, 0:1024])
            nc.scalar.dma_start(out=bias_t, in_=bias_d[:, :])
            nc.sync.dma_start(out=vt8_t[:, :3, 512:], in_=vt8_d[:, :3, 512:])
            nc.scalar.dma_start(out=vt8_t[:, 3:, 512:], in_=vt8_d[:, 3:, 512:])
            nc.sync.dma_start(
                out=tt8_t[0][:, :3, 1024:], in_=tt8_d[0, :, :3, 1024:]
            )
            nc.scalar.dma_start(
                out=tt8_t[0][:, 3:, 1024:], in_=tt8_d[0, :, 3:, 1024:]
            )
            nc.scalar.dma_start(out=wln_t[0][:, 1024:], in_=wln_d[0, :, 1024:])
            # strip 1 on sync, strip 3 on scalar, strip 2 split across both
            nc.sync.dma_start(out=tt8_t[1][:, :3, :], in_=tt8_d[1, :, :3, :])
            nc.sync.dma_start(out=tt8_t[1][:, 3:, :], in_=tt8_d[1, :, 3:, :])
            nc.sync.dma_start(out=wln_t[1], in_=wln_d[1, :, :])
            nc.scalar.dma_start(out=tt8_t[3][:, :3, :], in_=tt8_d[3, :, :3, :])
            nc.scalar.dma_start(out=tt8_t[3][:, 3:, :], in_=tt8_d[3, :, 3:, :])
            nc.scalar.dma_start(out=wln_t[3], in_=wln_d[3, :, :])
            nc.sync.dma_start(out=tt8_t[2][:, :3, :], in_=tt8_d[2, :, :3, :])
            nc.scalar.dma_start(out=tt8_t[2][:, 3:, :], in_=tt8_d[2, :, 3:, :])
            nc.sync.dma_start(out=wln_t[2], in_=wln_d[2, :, :])

            # preload the Exp/Ln ACT table during the DMA prologue
            scratch = consts.tile([128, 1], dt.float32, name="scratch")
            nc.vector.memset(scratch[:, :], 1.0)
            nc.scalar.activation(
                scratch[:, :], scratch[:, :], mybir.ActivationFunctionType.Exp
            )
            nc.vector.memset(rowparts_t[:, :], 0.0)

            # Dummy matmuls warm the HAM clock gate to 2.4 GHz while the
            # prologue DMA lands. warm_w memset comes first on its queue so
            # warmups start at ~1us.
            warm_w = consts.tile([128, 64], dt.bfloat16, name="warm_w")
            nc.vector.memset(warm_w[:, :], 0.0)
            pm_warm = mmps.tile([128, 512], dt.float32, name="pmw", tag="pm")
            for _ in range(95):
                nc.tensor.matmul(
                    pm_warm[:1, :64],
                    warm_w[:, 0:1],
                    warm_w[:, :],
                    start=True,
                    stop=True,
                )

            # Chunk schedule: first four m-tiles of strip 0 are half width
            # so fill-phase PE/ACT round trips stay short; the very last
            # chunk runs as four 512-wide pieces so the tail chain
            # (poly/exp/add/DMA) pipelines instead of serializing at 2048
            # width. (nq, m, lo, hi, rowparts slot)
            chunks = []
            for nq in range(NQ):
                for m in range(MT):
                    if nq == 0 and m == 0:
                        # m0 runs as two halves: the left one only needs
                        # the first half of strip 0 / wln 0, so compute
                        # starts as soon as ~0.5MB of input has landed
                        chunks.append((nq, m, 0, 1024, 32))
                        chunks.append((nq, m, 1024, 2048, 33))
                    else:
                        chunks.append((nq, m, 0, 2048, m * NQ + nq))



            for nq, m, lo, hi, idx in chunks:
                ms = slice(m * 128, (m + 1) * 128)
                width = hi - lo
                pm = mmps.tile([128, width], dt.float32, name="pm", tag="pm")
                for g in range(width // 512):
                    gs = slice(lo + g * 512, lo + (g + 1) * 512)
                    ps = pm[:, g * 512 : (g + 1) * 512]
                    for kp in range(KT // 2):
                        sp = slice(2 * kp, 2 * kp + 2)
                        nc.tensor.matmul(
                            ps,
                            vt8_t[:, sp, ms],
                            tt8_t[nq][:, sp, gs],
                            start=(kp == 0),
                            stop=(kp == KT // 2 - 1),
                            perf_mode=DR,
                        )
                # x = cubic(P) + lnW  (one fused custom DVE pass, frees PSUM)
                xt = xpool.tile([128, width], dt.float16, name="xt", tag="xt")
                nc.vector._custom_dve(
                    _POLY3W,
                    out=xt[:, :width],
                    in0=pm[:, :],
                    in1=wln_t[nq][:, lo:hi],
                    s0=float(a1),
                    s1=float(a2),
                    imm2=float(a3),
                )
                # et = exp(x + (A_i - SA)); accum_out = row partial sums
                cs = slice(nq * 2048 + lo, nq * 2048 + hi)
                if m == 0:
                    # first m-tile of a strip: Exp writes the column
                    # accumulator slice directly (no memset, no add)
                    nc.scalar.activation(
                        colacc[:, cs],
                        xt[:, :width],
                        mybir.ActivationFunctionType.Exp,
                        bias=bias_t[:, m : m + 1],
                        scale=1.0,
                        accum_out=rowparts_t[:, idx : idx + 1],
                    )
                else:
                    et = epool.tile([128, width], dt.float16, name="et", tag="et")
                    nc.scalar.activation(
                        et[:, :width],
                        xt[:, :width],
                        mybir.ActivationFunctionType.Exp,
                        bias=bias_t[:, m : m + 1],
                        scale=1.0,
                        accum_out=rowparts_t[:, idx : idx + 1],
                    )
                    if m < MT - 1:
                        # split ~60/40 by column: GPSIMD owns the left
                        # part, DVE the right, so the per-strip chains are
                        # independent and neither engine paces the other
                        cut = (width * 5 // 8) // 128 * 128
                        for eng, p0, p1 in (
                            (nc.gpsimd, 0, cut),
                            (nc.vector, cut, width),
                        ):
                            cs_h = slice(
                                nq * 2048 + lo + p0, nq * 2048 + lo + p1
                            )
                            eng.tensor_tensor(
                                colacc[:, cs_h],
                                colacc[:, cs_h],
                                et[:, p0:p1],
                                mybir.AluOpType.add,
                            )
                    else:
                        # strip end: split halves so the colsum DMA of
                        # half 0 overlaps the add of half 1
                        for hh in range(2):
                            cs_h = slice(
                                nq * 2048 + lo + hh * width // 2,
                                nq * 2048 + lo + (hh + 1) * width // 2,
                            )
                            nc.vector.tensor_tensor(
                                colacc[:, cs_h],
                                colacc[:, cs_h],
                                et[:, hh * width // 2 : (hh + 1) * width // 2],
                                mybir.AluOpType.add,
                            )
                            nc.sync.dma_start(
                                out=colsum_d[:, cs_h], in_=colacc[:, cs_h]
                            )

            nc.sync.dma_start(out=rowparts_d[:, :], in_=rowparts_t)

    nc.compile()
    return nc


def _host_prep(v, t, c_val):
    """fp64 host-side constants: diag logits, normalized fp8 operands."""
    v64 = np.asarray(v, np.float64)
    t64 = np.asarray(t, np.float64)
    inv_c = 1.0 / c_val
    k_eff = inv_c**0.5 / TEMPERATURE

    v_time = np.sqrt(inv_c + np.einsum("nd,nd->n", v64, v64))
    t_time = np.sqrt(inv_c + np.einsum("nd,nd->n", t64, t64))
    diag_dot = np.einsum("nd,nd->n", v64, t64)
    diag_arg = np.maximum(c_val * (v_time * t_time - diag_dot), 1.0 + EPS)
    a = -k_eff * np.arccosh(diag_arg)  # exact diag logits

    vn = (v64 / v_time[:, None] * FSCALE).astype(np.float32)
    tn = (t64 / t_time[:, None] * FSCALE).astype(np.float32)
    v8 = vn.astype(fp8)
    t8 = tn.astype(fp8)
    # [p, subtile, col] layout: element [p, s, j] = x[col j, feature s*128+p]
    vt8 = np.ascontiguousarray(v8.T.reshape(KT, 128, N).transpose(1, 0, 2))
    tt8_full = t8.T.reshape(KT, 128, N).transpose(1, 0, 2)  # [p, s, j]
    # strip-major [strip, p, subtile, j-within-strip]
    tt8 = np.ascontiguousarray(
        tt8_full.reshape(128, KT, NQ, 2048).transpose(2, 0, 1, 3)
    )

    A = -k_eff * (LN2 + np.log(c_val) + np.log(v_time))  # row factor
    B = -k_eff * np.log(t_time)  # col factor
    maxB = float(B.max())
    wln16 = (B - maxB).astype(fp16)  # device adds this inside the exp arg
    wln = np.ascontiguousarray(
        np.broadcast_to(wln16.reshape(NQ, 1, 2048), (NQ, 128, 2048))
    )

    # fit-range estimate for the cubic: sample 1/32 of v rows against all
    # t, take 1.3x margin, snap to a 0.02 grid (program cache stability)
    dsamp = (vn[::32] / FSCALE) @ (tn / FSCALE).T
    Rfit = float(np.abs(dsamp).max()) * 1.3
    Rfit = min(max(np.ceil(Rfit * 50.0) / 50.0, 0.10), 0.90)

    return a, k_eff, vt8, tt8, wln, A, B, maxB, wln16, Rfit


last_run_info = {}


def kernel(v_hyp, t_hyp, c, _trace=False):
    c_val = float(np.asarray(c))
    a, k_eff, vt8, tt8, wln, A, B, maxB, wln16, Rfit = _host_prep(
        v_hyp, t_hyp, c_val
    )

    key = (c_val, Rfit)
    if key not in _program_cache:
        _program_cache[key] = _build_program(c_val, Rfit)
    nc = _program_cache[key]
    c0 = _cubic_fit(Rfit)[0]

    SA = np.array([A[k * R : (k + 1) * R].max() for k in range(NCORES)])
    in_maps = []
    for k in range(NCORES):
        rows = slice(k * R, (k + 1) * R)
        # bias[p, m] = (A_i - SA) + k*c0 for row i = k*R + m*128 + p
        bias_mat = (
            (A[rows] - SA[k] + k_eff * c0).reshape(MT, 128).T.astype(np.float32)
        )
        in_maps.append(
            {
                "vt8": np.ascontiguousarray(vt8[:, :, rows]),
                "tt8": tt8,
                "wln": wln,
                "bias": np.ascontiguousarray(bias_mat),
            }
        )

    def _aggregate_rowsums(rp):
        # [128, 44]: 32 (m, nq) slots + 8 half-chunk slots for (nq0, m<4)
        # + 4 quarter-chunk slots for the (nq3, m7) finale; the unused
        # normal slots are zeroed on device.
        rp_pm = rp[:, : MT * NQ].reshape(128, MT, NQ).sum(axis=2)  # [p, m]
        for m in range(4):
            rp_pm[:, m] += rp[:, 32 + 2 * m] + rp[:, 33 + 2 * m]
        rp_pm[:, MT - 1] += rp[:, 40:44].sum(axis=1)
        return rp_pm

    # Rare first-execution flake has been observed to return garbage once;
    # outputs are cheap to validate (row sums must be finite and positive),
    # so retry a couple of times if that happens.
    for attempt in range(3):
        res = run_bass_kernel_spmd(nc, in_maps, list(range(NCORES)), trace=_trace)
        last_run_info["results"] = res
        results = res.results
        ok = all(
            np.all(np.isfinite(results[k]["rowparts"]))
            and np.all(
                _aggregate_rowsums(results[k]["rowparts"].astype(np.float64)) > 0
            )
            and np.all(np.isfinite(results[k]["colsum"].astype(np.float32)))
            for k in range(NCORES)
        )
        if ok:
            break

    # device row sums are sum_j exp(x_ij - SA_k - maxB)
    rowLSE = np.empty(N, np.float64)
    colsum_parts = np.empty((NCORES, N), np.float64)
    for k in range(NCORES):
        rp_pm = _aggregate_rowsums(results[k]["rowparts"].astype(np.float64))
        rows = slice(k * R, (k + 1) * R)
        rowLSE[rows] = np.log(rp_pm.T.reshape(R)) + (SA[k] + maxB)
        colsum_parts[k] = results[k]["colsum"].astype(np.float64).sum(axis=0)

    loss_v2t = np.mean(rowLSE - a)
    M0 = SA.max()
    # wln16 rides inside the device exponent, so col sums are already
    # complete shifted-exp column sums
    col = (colsum_parts * np.exp(SA - M0)[:, None]).sum(axis=0)
    colLSE = np.log(col) + M0 + maxB
    loss_t2v = np.mean(colLSE - a)
    return np.asarray(0.5 * (loss_v2t + loss_t2v), dtype=np.float32)


# revision 32
# speedup vs baseline: 1.1982x; 1.1982x over previous
"""Trainium2 Bass kernel for nn_DiscriminativeAlignmentLoss.

loss = 0.5*(CE_row + CE_col) over logits = -dist/T,
dist = (1/sqrt(c)) * arccosh(c*(v_time*t_time - v.t))   (Lorentz pairwise)

Strategy (8 cores, data parallel over v rows), v3 "normalized poly" scheme:
  - Host normalizes both sides by their Lorentz time components:
    v' = 32*v/vt_i, t' = 32*t/tt_j (fp8).  PSUM then holds P = 1024*d'
    with d' = (v.t)/(vt_i*tt_j), |d'| <~ 0.2, and the Lorentz arg factors:
      logit = A_i + B_j - k*ln(1-d'),  A_i = -k*(ln2+ln c+ln vt_i),
                                       B_j = -k*ln tt_j.
    This kills the baseline's rank-4 "time product" tail matmul: PE does
    exactly 3 fp8 DoubleRow matmuls (K=768) per 512-col group (82us/core).
  - A single CUSTOM DVE op (6 ALU stages, 1 pass) computes the whole
    exp argument from PSUM:  x = Horner_cubic(P) + lnW_j
    where the cubic is the minimax fit of -k*ln(1-d') over the observed
    d' range and lnW_j = B_j - maxB streams in as Src1 (fp16, replicated
    across partitions).  This replaces BOTH the baseline's ACT Ln pass
    (the kernel was ScalarE-bound at 80%) and any separate per-column
    weighting pass.
  - ACT does ONE Exp per chunk: et = exp(x + (A_i - SA)) = full shifted
    exp matrix; its accum_out gives row partial sums for free.
  - Column sums are plain fp16 adds (DVE 2x / GPSIMD), m==0 chunks write
    the accumulator directly from the Exp.  All shift/log arithmetic and
    the 128-partition reduction happen on host in fp64.
"""

import numpy as np
import ml_dtypes

import concourse.bass as bass  # noqa: F401  (registers AP machinery)
import concourse.tile as tile
from concourse import bacc, mybir
from concourse import hw_specs as _hw_specs
from concourse.bass_utils import run_bass_kernel_spmd
import concourse.dve_ops as _dve_ops
from concourse.dve_ops import DveOp as _DveOp
from concourse.dve_spec import Spec as _Spec, Src0, Src1, C0, C1, C2
from concourse.dve_spec import lower as _dve_lower
from concourse.dve_uop import DveOpSpec as _DveOpSpec

# The act-table insertion pass resolves each activation to the FIRST set
# containing its function. Keep Exp/Ln pinned to the combined set so a
# single ACT_TABLE_LOAD serves the whole kernel.
_orig_get_activation_tables = _hw_specs.get_activation_tables


def _patched_get_activation_tables(arch):
    tables = _orig_get_activation_tables(arch)
    drop = {mybir.ActivationFunctionType.Ln, mybir.ActivationFunctionType.Exp}
    return {
        name: (funcs if name == "natural_log_exp_and_others" else funcs - drop)
        for name, funcs in tables.items()
    }


bacc.get_activation_tables = _patched_get_activation_tables


# --- custom DVE op: x = ((P*c3 + c2)*P + c1)*P + lnW  (Horner, 6 stages) ---
def _poly3w_ref(in0, in1, s0, s1, imm2):
    x = in0.astype(np.float32)
    return ((x * imm2 + s1) * x + s0) * x + in1


def _register_poly3w():
    if "POLY3W_ANT" in _dve_ops._SUB_OPCODE_FOR_NAME:
        for op in _dve_ops.OPS:
            if op.name == "POLY3W_ANT":
                return op
    spec = _Spec(
        body=((Src0 * C2 + C1) * Src0 + C0) * Src0 + Src1,
        reference=_poly3w_ref,
    )
    shas = {
        v: _DveOpSpec(
            name="POLY3W_ANT", opcode=0, uops=_dve_lower(spec, ver=v), rd1_en=True
        ).sha(v)
        for v in ("v3", "v4")
    }
    op = _DveOp("POLY3W_ANT", spec, subdim=False, uops_sha=shas)
    _dve_ops.OPS.append(op)
    _dve_ops._SUB_OPCODE_FOR_NAME[op.name] = (
        _dve_ops._CUSTOM_DVE_ROW_BASE + len(_dve_ops.OPS) - 1
    )
    return op


_POLY3W = _register_poly3w()

N = 8192
D = 768
NCORES = 8
R = N // NCORES  # 1024 rows per core
MT = 8  # 128-row m-tiles per core
NQ = 4  # 2048-column chunks
KT = 6  # 128-row K subtiles (768 = 6*128)
TEMPERATURE = 0.07
EPS = 1e-6
LN2 = float(np.log(2.0))
FSCALE = 32.0  # fp8 pre-scale of the normalized operands (power of 2)
PSC = FSCALE * FSCALE  # PSUM = PSC * d'
bf16 = ml_dtypes.bfloat16
fp8 = ml_dtypes.float8_e4m3
fp16 = np.float16
dt = mybir.dt

# colacc adds: m-tiles whose add runs on GPSIMD (rest on DVE at 2x fp16).
# m==0 writes direct from the Exp, m==7 stays on DVE to keep the strip-end
# DMA chain short.
GP_ADD_MS = (1, 3, 5)

_program_cache = {}


def _cubic_fit(Rfit):
    """Minimax cubic fit of -ln(1-x) ~ c0+c1 x+c2 x^2+c3 x^3 on [-R, R]."""
    xs = np.cos(np.pi * (np.arange(4000) + 0.5) / 4000) * Rfit
    coef = np.polynomial.chebyshev.chebfit(xs, -np.log1p(-xs), 3)
    return [float(z) for z in np.polynomial.chebyshev.cheb2poly(coef)]


def _build_program(c: float, Rfit: float):
    """Build + compile the per-core Bass program (same on all 8 cores)."""
    k_eff = (1.0 / c) ** 0.5 / TEMPERATURE
    c0, c1, c2, c3 = _cubic_fit(Rfit)
    a1 = k_eff * c1 / PSC
    a2 = k_eff * c2 / (PSC * PSC)
    a3 = k_eff * c3 / (PSC * PSC * PSC)

    nc = bacc.Bacc(
        "TRN2",
        target_bir_lowering=False,
        debug=False,
        enable_asserts=False,
        num_devices=NCORES,
    )

    vt8_d = nc.dram_tensor("vt8", [128, KT, R], dt.float8e4, kind="ExternalInput")
    tt8_d = nc.dram_tensor(
        "tt8", [NQ, 128, KT, 2048], dt.float8e4, kind="ExternalInput"
    )
    wln_d = nc.dram_tensor("wln", [NQ, 128, 2048], dt.float16, kind="ExternalInput")
    bias_d = nc.dram_tensor("bias", [128, MT], dt.float32, kind="ExternalInput")
    rowparts_d = nc.dram_tensor(
        "rowparts", [128, MT * NQ + 12], dt.float32, kind="ExternalOutput"
    )
    colsum_d = nc.dram_tensor("colsum", [128, N], dt.float16, kind="ExternalOutput")

    DR = mybir.MatmulPerfMode.DoubleRow

    with tile.TileContext(nc) as tc:
        with (
            tc.tile_pool(name="consts", bufs=1) as consts,
            tc.tile_pool(name="xpool", bufs=3) as xpool,
            tc.tile_pool(name="epool", bufs=3) as epool,
            tc.tile_pool(name="mmps", bufs=2, space="PSUM") as mmps,
        ):
            tt8_t = [
                consts.tile([128, KT, 2048], dt.float8e4, name=f"tt8_{s}")
                for s in range(NQ)
            ]
            wln_t = [
                consts.tile([128, 2048], dt.float16, name=f"wln_{s}")
                for s in range(NQ)
            ]
            vt8_t = consts.tile([128, KT, R], dt.float8e4, name="vt8_t")
            bias_t = consts.tile([128, MT], dt.float32, name="bias_t")
            rowparts_t = consts.tile(
                [128, MT * NQ + 12], dt.float32, name="rowparts_t"
            )
            colacc = consts.tile([128, N], dt.float16, name="colacc")

            # Input DMAs ride both hardware DGE rings (sync + scalar), in
            # need order at fine grain so the first fill chunks (m0-m3,
            # cols 0:1024 of strip 0) can start ~5us earlier than a bulk
            # strip load would allow: any delay to the first real matmuls
            # lets the HAM clock gate re-throttle the PE after the warmup
            # stream ends. Later strips are split so each arrives well
            # before its chunks run.
            nc.sync.dma_start(out=vt8_t[:, :3, 0:512], in_=vt8_d[:, :3, 0:512])
            nc.scalar.dma_start(out=vt8_t[:, 3:, 0:512], in_=vt8_d[:, 3:, 0:512])
            nc.sync.dma_start(
                out=tt8_t[0][:, :3, 0:1024], in_=tt8_d[0, :, :3, 0:1024]
            )
            nc.scalar.dma_start(
                out=tt8_t[0][:, 3:, 0:1024], in_=tt8_d[0, :, 3:, 0:1024]
            )
            nc.scalar.dma_start(out=wln_t[0][:, 0:1024], in_=wln_d[0, :, 0:1024])
            nc.scalar.dma_start(out=bias_t, in_=bias_d[:, :])
            nc.sync.dma_start(out=vt8_t[:, :3, 512:], in_=vt8_d[:, :3, 512:])
            nc.scalar.dma_start(out=vt8_t[:, 3:, 512:], in_=vt8_d[:, 3:, 512:])
            nc.sync.dma_start(
                out=tt8_t[0][:, :3, 1024:], in_=tt8_d[0, :, :3, 1024:]
            )
            nc.scalar.dma_start(
                out=tt8_t[0][:, 3:, 1024:], in_=tt8_d[0, :, 3:, 1024:]
            )
            nc.scalar.dma_start(out=wln_t[0][:, 1024:], in_=wln_d[0, :, 1024:])
            # strip 1 on sync, strip 3 on scalar, strip 2 split across both
            nc.sync.dma_start(out=tt8_t[1][:, :3, :], in_=tt8_d[1, :, :3, :])
            nc.sync.dma_start(out=tt8_t[1][:, 3:, :], in_=tt8_d[1, :, 3:, :])
            nc.sync.dma_start(out=wln_t[1], in_=wln_d[1, :, :])
            nc.scalar.dma_start(out=tt8_t[3][:, :3, :], in_=tt8_d[3, :, :3, :])
            nc.scalar.dma_start(out=tt8_t[3][:, 3:, :], in_=tt8_d[3, :, 3:, :])
            nc.scalar.dma_start(out=wln_t[3], in_=wln_d[3, :, :])
            nc.sync.dma_start(out=tt8_t[2][:, :3, :], in_=tt8_d[2, :, :3, :])
            nc.scalar.dma_start(out=tt8_t[2][:, 3:, :], in_=tt8_d[2, :, 3:, :])
            nc.sync.dma_start(out=wln_t[2], in_=wln_d[2, :, :])

            # preload the Exp/Ln ACT table during the DMA prologue
            scratch = consts.tile([128, 1], dt.float32, name="scratch")
            nc.vector.memset(scratch[:, :], 1.0)
            nc.scalar.activation(
                scratch[:, :], scratch[:, :], mybir.ActivationFunctionType.Exp
            )
            nc.vector.memset(rowparts_t[:, :], 0.0)

            # Dummy matmuls warm the HAM clock gate to 2.4 GHz while the
            # prologue DMA lands. warm_w memset comes first on its queue so
            # warmups start at ~1us.
            warm_w = consts.tile([128, 64], dt.bfloat16, name="warm_w")
            nc.vector.memset(warm_w[:, :], 0.0)
            pm_warm = mmps.tile([128, 512], dt.float32, name="pmw", tag="pm")
            for _ in range(95):
                nc.tensor.matmul(
                    pm_warm[:1, :64],
                    warm_w[:, 0:1],
                    warm_w[:, :],
                    start=True,
                    stop=True,
                )

            # Chunk schedule: first four m-tiles of strip 0 are half width
            # so fill-phase PE/ACT round trips stay short; the very last
            # chunk runs as four 512-wide pieces so the tail chain
            # (poly/exp/add/DMA) pipelines instead of serializing at 2048
            # width. (nq, m, lo, hi, rowparts slot)
            chunks = []
            for nq in range(NQ):
                for m in range(MT):
                    if nq == 0 and m == 0:
                        # m0 runs as two halves: the left one only needs
                        # the first half of strip 0 / wln 0, so compute
                        # starts as soon as ~0.5MB of input has landed
                        chunks.append((nq, m, 0, 1024, 32))
                        chunks.append((nq, m, 1024, 2048, 33))
                    else:
                        chunks.append((nq, m, 0, 2048, m * NQ + nq))



            # colacc adds are emitted one chunk late so neither add queue
            # head-of-line blocks on the Exp that produces its input
            pending = []

            def _flush_pending():
                while pending:
                    pending.pop(0)()

            for nq, m, lo, hi, idx in chunks:
                ms = slice(m * 128, (m + 1) * 128)
                width = hi - lo
                pm = mmps.tile([128, width], dt.float32, name="pm", tag="pm")
                for g in range(width // 512):
                    gs = slice(lo + g * 512, lo + (g + 1) * 512)
                    ps = pm[:, g * 512 : (g + 1) * 512]
                    for kp in range(KT // 2):
                        sp = slice(2 * kp, 2 * kp + 2)
                        nc.tensor.matmul(
                            ps,
                            vt8_t[:, sp, ms],
                            tt8_t[nq][:, sp, gs],
                            start=(kp == 0),
                            stop=(kp == KT // 2 - 1),
                            perf_mode=DR,
                        )
                # x = cubic(P) + lnW  (one fused custom DVE pass, frees PSUM)
                xt = xpool.tile([128, width], dt.float16, name="xt", tag="xt")
                nc.vector._custom_dve(
                    _POLY3W,
                    out=xt[:, :width],
                    in0=pm[:, :],
                    in1=wln_t[nq][:, lo:hi],
                    s0=float(a1),
                    s1=float(a2),
                    imm2=float(a3),
                )
                # et = exp(x + (A_i - SA)); accum_out = row partial sums
                cs = slice(nq * 2048 + lo, nq * 2048 + hi)
                if m == 0:
                    # first m-tile of a strip: Exp writes the column
                    # accumulator slice directly (no memset, no add)
                    nc.scalar.activation(
                        colacc[:, cs],
                        xt[:, :width],
                        mybir.ActivationFunctionType.Exp,
                        bias=bias_t[:, m : m + 1],
                        scale=1.0,
                        accum_out=rowparts_t[:, idx : idx + 1],
                    )
                    _flush_pending()
                else:
                    et = epool.tile([128, width], dt.float16, name="et", tag="et")
                    nc.scalar.activation(
                        et[:, :width],
                        xt[:, :width],
                        mybir.ActivationFunctionType.Exp,
                        bias=bias_t[:, m : m + 1],
                        scale=1.0,
                        accum_out=rowparts_t[:, idx : idx + 1],
                    )
                    _flush_pending()

                    def _mk_add(nq=nq, m=m, lo=lo, et=et, w=width):
                        def _emit():
                            if m < MT - 1:
                                # split ~60/40 by column: GPSIMD owns the
                                # left part, DVE the right, so the
                                # per-strip chains are independent
                                cut = (w * 5 // 8) // 128 * 128
                                parts = (
                                    (nc.gpsimd, 0, cut),
                                    (nc.vector, cut, w),
                                )
                            else:
                                parts = (
                                    (nc.vector, 0, w // 2),
                                    (nc.vector, w // 2, w),
                                )
                            for eng, p0, p1 in parts:
                                cs_h = slice(
                                    nq * 2048 + lo + p0, nq * 2048 + lo + p1
                                )
                                eng.tensor_tensor(
                                    colacc[:, cs_h],
                                    colacc[:, cs_h],
                                    et[:, p0:p1],
                                    mybir.AluOpType.add,
                                )
                                if m == MT - 1:
                                    nc.sync.dma_start(
                                        out=colsum_d[:, cs_h],
                                        in_=colacc[:, cs_h],
                                    )

                        return _emit

                    pending.append(_mk_add())
            _flush_pending()

            nc.sync.dma_start(out=rowparts_d[:, :], in_=rowparts_t)

    nc.compile()
    return nc


def _host_prep(v, t, c_val):
    """fp64 host-side constants: diag logits, normalized fp8 operands."""
    v64 = np.asarray(v, np.float64)
    t64 = np.asarray(t, np.float64)
    inv_c = 1.0 / c_val
    k_eff = inv_c**0.5 / TEMPERATURE

    v_time = np.sqrt(inv_c + np.einsum("nd,nd->n", v64, v64))
    t_time = np.sqrt(inv_c + np.einsum("nd,nd->n", t64, t64))
    diag_dot = np.einsum("nd,nd->n", v64, t64)
    diag_arg = np.maximum(c_val * (v_time * t_time - diag_dot), 1.0 + EPS)
    a = -k_eff * np.arccosh(diag_arg)  # exact diag logits

    vn = (v64 / v_time[:, None] * FSCALE).astype(np.float32)
    tn = (t64 / t_time[:, None] * FSCALE).astype(np.float32)
    v8 = vn.astype(fp8)
    t8 = tn.astype(fp8)
    # [p, subtile, col] layout: element [p, s, j] = x[col j, feature s*128+p]
    vt8 = np.ascontiguousarray(v8.T.reshape(KT, 128, N).transpose(1, 0, 2))
    tt8_full = t8.T.reshape(KT, 128, N).transpose(1, 0, 2)  # [p, s, j]
    # strip-major [strip, p, subtile, j-within-strip]
    tt8 = np.ascontiguousarray(
        tt8_full.reshape(128, KT, NQ, 2048).transpose(2, 0, 1, 3)
    )

    A = -k_eff * (LN2 + np.log(c_val) + np.log(v_time))  # row factor
    B = -k_eff * np.log(t_time)  # col factor
    maxB = float(B.max())
    wln16 = (B - maxB).astype(fp16)  # device adds this inside the exp arg
    wln = np.ascontiguousarray(
        np.broadcast_to(wln16.reshape(NQ, 1, 2048), (NQ, 128, 2048))
    )

    # fit-range estimate for the cubic: sample 1/32 of v rows against all
    # t, take 1.3x margin, snap to a 0.02 grid (program cache stability)
    dsamp = (vn[::32] / FSCALE) @ (tn / FSCALE).T
    Rfit = float(np.abs(dsamp).max()) * 1.3
    Rfit = min(max(np.ceil(Rfit * 50.0) / 50.0, 0.10), 0.90)

    return a, k_eff, vt8, tt8, wln, A, B, maxB, wln16, Rfit


last_run_info = {}


def kernel(v_hyp, t_hyp, c, _trace=False):
    c_val = float(np.asarray(c))
    a, k_eff, vt8, tt8, wln, A, B, maxB, wln16, Rfit = _host_prep(
        v_hyp, t_hyp, c_val
    )

    key = (c_val, Rfit)
    if key not in _program_cache:
        _program_cache[key] = _build_program(c_val, Rfit)
    nc = _program_cache[key]
    c0 = _cubic_fit(Rfit)[0]

    SA = np.array([A[k * R : (k + 1) * R].max() for k in range(NCORES)])
    in_maps = []
    for k in range(NCORES):
        rows = slice(k * R, (k + 1) * R)
        # bias[p, m] = (A_i - SA) + k*c0 for row i = k*R + m*128 + p
        bias_mat = (
            (A[rows] - SA[k] + k_eff * c0).reshape(MT, 128).T.astype(np.float32)
        )
        in_maps.append(
            {
                "vt8": np.ascontiguousarray(vt8[:, :, rows]),
                "tt8": tt8,
                "wln": wln,
                "bias": np.ascontiguousarray(bias_mat),
            }
        )

    def _aggregate_rowsums(rp):
        # [128, 44]: 32 (m, nq) slots + 8 half-chunk slots for (nq0, m<4)
        # + 4 quarter-chunk slots for the (nq3, m7) finale; the unused
        # normal slots are zeroed on device.
        rp_pm = rp[:, : MT * NQ].reshape(128, MT, NQ).sum(axis=2)  # [p, m]
        for m in range(4):
            rp_pm[:, m] += rp[:, 32 + 2 * m] + rp[:, 33 + 2 * m]
        rp_pm[:, MT - 1] += rp[:, 40:44].sum(axis=1)
        return rp_pm

    # Rare first-execution flake has been observed to return garbage once;
    # outputs are cheap to validate (row sums must be finite and positive),
    # so retry a couple of times if that happens.
    for attempt in range(3):
        res = run_bass_kernel_spmd(nc, in_maps, list(range(NCORES)), trace=_trace)
        last_run_info["results"] = res
        results = res.results
        ok = all(
            np.all(np.isfinite(results[k]["rowparts"]))
            and np.all(
                _aggregate_rowsums(results[k]["rowparts"].astype(np.float64)) > 0
            )
            and np.all(np.isfinite(results[k]["colsum"].astype(np.float32)))
            for k in range(NCORES)
        )
        if ok:
            break

    # device row sums are sum_j exp(x_ij - SA_k - maxB)
    rowLSE = np.empty(N, np.float64)
    colsum_parts = np.empty((NCORES, N), np.float64)
    for k in range(NCORES):
        rp_pm = _aggregate_rowsums(results[k]["rowparts"].astype(np.float64))
        rows = slice(k * R, (k + 1) * R)
        rowLSE[rows] = np.log(rp_pm.T.reshape(R)) + (SA[k] + maxB)
        colsum_parts[k] = results[k]["colsum"].astype(np.float64).sum(axis=0)

    loss_v2t = np.mean(rowLSE - a)
    M0 = SA.max()
    # wln16 rides inside the device exponent, so col sums are already
    # complete shifted-exp column sums
    col = (colsum_parts * np.exp(SA - M0)[:, None]).sum(axis=0)
    colLSE = np.log(col) + M0 + maxB
    loss_t2v = np.mean(colLSE - a)
    return np.asarray(0.5 * (loss_v2t + loss_t2v), dtype=np.float32)


# revision 34
# speedup vs baseline: 1.2039x; 1.0047x over previous
"""Trainium2 Bass kernel for nn_DiscriminativeAlignmentLoss.

loss = 0.5*(CE_row + CE_col) over logits = -dist/T,
dist = (1/sqrt(c)) * arccosh(c*(v_time*t_time - v.t))   (Lorentz pairwise)

Strategy (8 cores, data parallel over v rows), v3 "normalized poly" scheme:
  - Host normalizes both sides by their Lorentz time components:
    v' = 32*v/vt_i, t' = 32*t/tt_j (fp8).  PSUM then holds P = 1024*d'
    with d' = (v.t)/(vt_i*tt_j), |d'| <~ 0.2, and the Lorentz arg factors:
      logit = A_i + B_j - k*ln(1-d'),  A_i = -k*(ln2+ln c+ln vt_i),
                                       B_j = -k*ln tt_j.
    This kills the baseline's rank-4 "time product" tail matmul: PE does
    exactly 3 fp8 DoubleRow matmuls (K=768) per 512-col group (82us/core).
  - A single CUSTOM DVE op (6 ALU stages, 1 pass) computes the whole
    exp argument from PSUM:  x = Horner_cubic(P) + lnW_j
    where the cubic is the minimax fit of -k*ln(1-d') over the observed
    d' range and lnW_j = B_j - maxB streams in as Src1 (fp16, replicated
    across partitions).  This replaces BOTH the baseline's ACT Ln pass
    (the kernel was ScalarE-bound at 80%) and any separate per-column
    weighting pass.
  - ACT does ONE Exp per chunk: et = exp(x + (A_i - SA)) = full shifted
    exp matrix; its accum_out gives row partial sums for free.
  - Column sums are plain fp16 adds (DVE 2x / GPSIMD), m==0 chunks write
    the accumulator directly from the Exp.  All shift/log arithmetic and
    the 128-partition reduction happen on host in fp64.
"""

import numpy as np
import ml_dtypes

import concourse.bass as bass  # noqa: F401  (registers AP machinery)
import concourse.tile as tile
from concourse import bacc, mybir
from concourse import hw_specs as _hw_specs
from concourse.bass_utils import run_bass_kernel_spmd
import concourse.dve_ops as _dve_ops
from concourse.dve_ops import DveOp as _DveOp
from concourse.dve_spec import Spec as _Spec, Src0, Src1, C0, C1, C2
from concourse.dve_spec import lower as _dve_lower
from concourse.dve_uop import DveOpSpec as _DveOpSpec

# The act-table insertion pass resolves each activation to the FIRST set
# containing its function. Keep Exp/Ln pinned to the combined set so a
# single ACT_TABLE_LOAD serves the whole kernel.
_orig_get_activation_tables = _hw_specs.get_activation_tables


def _patched_get_activation_tables(arch):
    tables = _orig_get_activation_tables(arch)
    drop = {mybir.ActivationFunctionType.Ln, mybir.ActivationFunctionType.Exp}
    return {
        name: (funcs if name == "natural_log_exp_and_others" else funcs - drop)
        for name, funcs in tables.items()
    }


bacc.get_activation_tables = _patched_get_activation_tables


# --- custom DVE op: x = ((P*c3 + c2)*P + c1)*P + lnW  (Horner, 6 stages) ---
def _poly3w_ref(in0, in1, s0, s1, imm2):
    x = in0.astype(np.float32)
    return ((x * imm2 + s1) * x + s0) * x + in1


def _register_poly3w():
    if "POLY3W_ANT" in _dve_ops._SUB_OPCODE_FOR_NAME:
        for op in _dve_ops.OPS:
            if op.name == "POLY3W_ANT":
                return op
    spec = _Spec(
        body=((Src0 * C2 + C1) * Src0 + C0) * Src0 + Src1,
        reference=_poly3w_ref,
    )
    shas = {
        v: _DveOpSpec(
            name="POLY3W_ANT", opcode=0, uops=_dve_lower(spec, ver=v), rd1_en=True
        ).sha(v)
        for v in ("v3", "v4")
    }
    op = _DveOp("POLY3W_ANT", spec, subdim=False, uops_sha=shas)
    _dve_ops.OPS.append(op)
    _dve_ops._SUB_OPCODE_FOR_NAME[op.name] = (
        _dve_ops._CUSTOM_DVE_ROW_BASE + len(_dve_ops.OPS) - 1
    )
    return op


_POLY3W = _register_poly3w()

N = 8192
D = 768
NCORES = 8
R = N // NCORES  # 1024 rows per core
MT = 8  # 128-row m-tiles per core
NQ = 4  # 2048-column chunks
KT = 6  # 128-row K subtiles (768 = 6*128)
TEMPERATURE = 0.07
EPS = 1e-6
LN2 = float(np.log(2.0))
FSCALE = 32.0  # fp8 pre-scale of the normalized operands (power of 2)
PSC = FSCALE * FSCALE  # PSUM = PSC * d'
bf16 = ml_dtypes.bfloat16
fp8 = ml_dtypes.float8_e4m3
fp16 = np.float16
dt = mybir.dt

# colacc adds: m-tiles whose add runs on GPSIMD (rest on DVE at 2x fp16).
# m==0 writes direct from the Exp, m==7 stays on DVE to keep the strip-end
# DMA chain short.
GP_ADD_MS = (1, 3, 5)

_program_cache = {}


def _cubic_fit(Rfit):
    """Minimax cubic fit of -ln(1-x) ~ c0+c1 x+c2 x^2+c3 x^3 on [-R, R]."""
    xs = np.cos(np.pi * (np.arange(4000) + 0.5) / 4000) * Rfit
    coef = np.polynomial.chebyshev.chebfit(xs, -np.log1p(-xs), 3)
    return [float(z) for z in np.polynomial.chebyshev.cheb2poly(coef)]


def _build_program(c: float, Rfit: float):
    """Build + compile the per-core Bass program (same on all 8 cores)."""
    k_eff = (1.0 / c) ** 0.5 / TEMPERATURE
    c0, c1, c2, c3 = _cubic_fit(Rfit)
    a1 = k_eff * c1 / PSC
    a2 = k_eff * c2 / (PSC * PSC)
    a3 = k_eff * c3 / (PSC * PSC * PSC)

    nc = bacc.Bacc(
        "TRN2",
        target_bir_lowering=False,
        debug=False,
        enable_asserts=False,
        num_devices=NCORES,
    )

    vt8_d = nc.dram_tensor("vt8", [128, KT, R], dt.float8e4, kind="ExternalInput")
    tt8_d = nc.dram_tensor(
        "tt8", [NQ, 128, KT, 2048], dt.float8e4, kind="ExternalInput"
    )
    wln_d = nc.dram_tensor("wln", [NQ, 128, 2048], dt.float16, kind="ExternalInput")
    bias_d = nc.dram_tensor("bias", [128, MT], dt.float32, kind="ExternalInput")
    rowparts_d = nc.dram_tensor(
        "rowparts", [128, MT * NQ + 12], dt.float32, kind="ExternalOutput"
    )
    colsum_d = nc.dram_tensor("colsum", [128, N], dt.float16, kind="ExternalOutput")

    DR = mybir.MatmulPerfMode.DoubleRow

    with tile.TileContext(nc) as tc:
        with (
            tc.tile_pool(name="consts", bufs=1) as consts,
            tc.tile_pool(name="xpool", bufs=3) as xpool,
            tc.tile_pool(name="epool", bufs=3) as epool,
            tc.tile_pool(name="mmps", bufs=2, space="PSUM") as mmps,
        ):
            tt8_t = [
                consts.tile([128, KT, 2048], dt.float8e4, name=f"tt8_{s}")
                for s in range(NQ)
            ]
            wln_t = [
                consts.tile([128, 2048], dt.float16, name=f"wln_{s}")
                for s in range(NQ)
            ]
            vt8_t = consts.tile([128, KT, R], dt.float8e4, name="vt8_t")
            bias_t = consts.tile([128, MT], dt.float32, name="bias_t")
            rowparts_t = consts.tile(
                [128, MT * NQ + 12], dt.float32, name="rowparts_t"
            )
            colacc = consts.tile([128, N], dt.float16, name="colacc")

            # Input DMAs ride both hardware DGE rings (sync + scalar), in
            # need order at fine grain so the first fill chunks (m0-m3,
            # cols 0:1024 of strip 0) can start ~5us earlier than a bulk
            # strip load would allow: any delay to the first real matmuls
            # lets the HAM clock gate re-throttle the PE after the warmup
            # stream ends. Later strips are split so each arrives well
            # before its chunks run.
            nc.sync.dma_start(out=vt8_t[:, :3, 0:512], in_=vt8_d[:, :3, 0:512])
            nc.scalar.dma_start(out=vt8_t[:, 3:, 0:512], in_=vt8_d[:, 3:, 0:512])
            nc.sync.dma_start(
                out=tt8_t[0][:, :3, 0:1024], in_=tt8_d[0, :, :3, 0:1024]
            )
            nc.scalar.dma_start(
                out=tt8_t[0][:, 3:, 0:1024], in_=tt8_d[0, :, 3:, 0:1024]
            )
            nc.scalar.dma_start(out=wln_t[0][:, 0:1024], in_=wln_d[0, :, 0:1024])
            nc.scalar.dma_start(out=bias_t, in_=bias_d[:, :])
            nc.sync.dma_start(out=vt8_t[:, :3, 512:], in_=vt8_d[:, :3, 512:])
            nc.scalar.dma_start(out=vt8_t[:, 3:, 512:], in_=vt8_d[:, 3:, 512:])
            nc.sync.dma_start(
                out=tt8_t[0][:, :3, 1024:], in_=tt8_d[0, :, :3, 1024:]
            )
            nc.scalar.dma_start(
                out=tt8_t[0][:, 3:, 1024:], in_=tt8_d[0, :, 3:, 1024:]
            )
            nc.scalar.dma_start(out=wln_t[0][:, 1024:], in_=wln_d[0, :, 1024:])
            # strip 1 on sync, strip 3 on scalar, strip 2 split across both
            nc.sync.dma_start(out=tt8_t[1][:, :3, :], in_=tt8_d[1, :, :3, :])
            nc.sync.dma_start(out=tt8_t[1][:, 3:, :], in_=tt8_d[1, :, 3:, :])
            nc.sync.dma_start(out=wln_t[1], in_=wln_d[1, :, :])
            nc.scalar.dma_start(out=tt8_t[3][:, :3, :], in_=tt8_d[3, :, :3, :])
            nc.scalar.dma_start(out=tt8_t[3][:, 3:, :], in_=tt8_d[3, :, 3:, :])
            nc.scalar.dma_start(out=wln_t[3], in_=wln_d[3, :, :])
            nc.sync.dma_start(out=tt8_t[2][:, :3, :], in_=tt8_d[2, :, :3, :])
            nc.scalar.dma_start(out=tt8_t[2][:, 3:, :], in_=tt8_d[2, :, 3:, :])
            nc.sync.dma_start(out=wln_t[2], in_=wln_d[2, :, :])

            # preload the Exp/Ln ACT table during the DMA prologue
            scratch = consts.tile([128, 1], dt.float32, name="scratch")
            nc.vector.memset(scratch[:, :], 1.0)
            nc.scalar.activation(
                scratch[:, :], scratch[:, :], mybir.ActivationFunctionType.Exp
            )
            nc.vector.memset(rowparts_t[:, :], 0.0)

            # Dummy matmuls warm the HAM clock gate to 2.4 GHz while the
            # prologue DMA lands. warm_w memset comes first on its queue so
            # warmups start at ~1us.
            warm_w = consts.tile([128, 64], dt.bfloat16, name="warm_w")
            nc.vector.memset(warm_w[:, :], 0.0)
            pm_warm = mmps.tile([128, 512], dt.float32, name="pmw", tag="pm")
            for _ in range(95):
                nc.tensor.matmul(
                    pm_warm[:1, :64],
                    warm_w[:, 0:1],
                    warm_w[:, :],
                    start=True,
                    stop=True,
                )

            # Chunk schedule: first four m-tiles of strip 0 are half width
            # so fill-phase PE/ACT round trips stay short; the very last
            # chunk runs as four 512-wide pieces so the tail chain
            # (poly/exp/add/DMA) pipelines instead of serializing at 2048
            # width. (nq, m, lo, hi, rowparts slot)
            chunks = []
            for nq in range(NQ):
                for m in range(MT):
                    if nq == 0 and m == 0:
                        # m0 runs as two halves: the left one only needs
                        # the first half of strip 0 / wln 0, so compute
                        # starts as soon as ~0.5MB of input has landed
                        chunks.append((nq, m, 0, 1024, 32))
                        chunks.append((nq, m, 1024, 2048, 33))
                    else:
                        chunks.append((nq, m, 0, 2048, m * NQ + nq))



            # colacc adds are emitted one chunk late so neither add queue
            # head-of-line blocks on the Exp that produces its input
            pending = []

            def _flush_pending():
                while pending:
                    pending.pop(0)()

            for nq, m, lo, hi, idx in chunks:
                ms = slice(m * 128, (m + 1) * 128)
                width = hi - lo
                # previous chunk's colacc adds go first: their inputs are
                # ready, so they fill the DVE/GPSIMD queues while this
                # chunk's matmuls run instead of queueing behind the poly
                _flush_pending()
                pm = mmps.tile([128, width], dt.float32, name="pm", tag="pm")
                for g in range(width // 512):
                    gs = slice(lo + g * 512, lo + (g + 1) * 512)
                    ps = pm[:, g * 512 : (g + 1) * 512]
                    for kp in range(KT // 2):
                        sp = slice(2 * kp, 2 * kp + 2)
                        nc.tensor.matmul(
                            ps,
                            vt8_t[:, sp, ms],
                            tt8_t[nq][:, sp, gs],
                            start=(kp == 0),
                            stop=(kp == KT // 2 - 1),
                            perf_mode=DR,
                        )
                # x = cubic(P) + lnW  (one fused custom DVE pass, frees PSUM)
                xt = xpool.tile([128, width], dt.float16, name="xt", tag="xt")
                nc.vector._custom_dve(
                    _POLY3W,
                    out=xt[:, :width],
                    in0=pm[:, :],
                    in1=wln_t[nq][:, lo:hi],
                    s0=float(a1),
                    s1=float(a2),
                    imm2=float(a3),
                )
                # et = exp(x + (A_i - SA)); accum_out = row partial sums
                cs = slice(nq * 2048 + lo, nq * 2048 + hi)
                if m == 0:
                    # first m-tile of a strip: Exp writes the column
                    # accumulator slice directly (no memset, no add)
                    nc.scalar.activation(
                        colacc[:, cs],
                        xt[:, :width],
                        mybir.ActivationFunctionType.Exp,
                        bias=bias_t[:, m : m + 1],
                        scale=1.0,
                        accum_out=rowparts_t[:, idx : idx + 1],
                    )
                else:
                    et = epool.tile([128, width], dt.float16, name="et", tag="et")
                    nc.scalar.activation(
                        et[:, :width],
                        xt[:, :width],
                        mybir.ActivationFunctionType.Exp,
                        bias=bias_t[:, m : m + 1],
                        scale=1.0,
                        accum_out=rowparts_t[:, idx : idx + 1],
                    )

                    def _mk_add(nq=nq, m=m, lo=lo, et=et, w=width):
                        def _emit():
                            if m < MT - 1:
                                # split ~60/40 by column: GPSIMD owns the
                                # left part, DVE the right, so the
                                # per-strip chains are independent
                                cut = (w * 5 // 8) // 128 * 128
                                parts = (
                                    (nc.gpsimd, 0, cut),
                                    (nc.vector, cut, w),
                                )
                            else:
                                parts = (
                                    (nc.vector, 0, w // 2),
                                    (nc.vector, w // 2, w),
                                )
                            for eng, p0, p1 in parts:
                                cs_h = slice(
                                    nq * 2048 + lo + p0, nq * 2048 + lo + p1
                                )
                                eng.tensor_tensor(
                                    colacc[:, cs_h],
                                    colacc[:, cs_h],
                                    et[:, p0:p1],
                                    mybir.AluOpType.add,
                                )
                                if m == MT - 1:
                                    nc.sync.dma_start(
                                        out=colsum_d[:, cs_h],
                                        in_=colacc[:, cs_h],
                                    )

                        return _emit

                    pending.append(_mk_add())
            _flush_pending()

            nc.sync.dma_start(out=rowparts_d[:, :], in_=rowparts_t)

    nc.compile()
    return nc


def _host_prep(v, t, c_val):
    """fp64 host-side constants: diag logits, normalized fp8 operands."""
    v64 = np.asarray(v, np.float64)
    t64 = np.asarray(t, np.float64)
    inv_c = 1.0 / c_val
    k_eff = inv_c**0.5 / TEMPERATURE

    v_time = np.sqrt(inv_c + np.einsum("nd,nd->n", v64, v64))
    t_time = np.sqrt(inv_c + np.einsum("nd,nd->n", t64, t64))
    diag_dot = np.einsum("nd,nd->n", v64, t64)
    diag_arg = np.maximum(c_val * (v_time * t_time - diag_dot), 1.0 + EPS)
    a = -k_eff * np.arccosh(diag_arg)  # exact diag logits

    vn = (v64 / v_time[:, None] * FSCALE).astype(np.float32)
    tn = (t64 / t_time[:, None] * FSCALE).astype(np.float32)
    v8 = vn.astype(fp8)
    t8 = tn.astype(fp8)
    # [p, subtile, col] layout: element [p, s, j] = x[col j, feature s*128+p]
    vt8 = np.ascontiguousarray(v8.T.reshape(KT, 128, N).transpose(1, 0, 2))
    tt8_full = t8.T.reshape(KT, 128, N).transpose(1, 0, 2)  # [p, s, j]
    # strip-major [strip, p, subtile, j-within-strip]
    tt8 = np.ascontiguousarray(
        tt8_full.reshape(128, KT, NQ, 2048).transpose(2, 0, 1, 3)
    )

    A = -k_eff * (LN2 + np.log(c_val) + np.log(v_time))  # row factor
    B = -k_eff * np.log(t_time)  # col factor
    maxB = float(B.max())
    wln16 = (B - maxB).astype(fp16)  # device adds this inside the exp arg
    wln = np.ascontiguousarray(
        np.broadcast_to(wln16.reshape(NQ, 1, 2048), (NQ, 128, 2048))
    )

    # fit-range estimate for the cubic: sample 1/32 of v rows against all
    # t, take 1.3x margin, snap to a 0.02 grid (program cache stability)
    dsamp = (vn[::32] / FSCALE) @ (tn / FSCALE).T
    Rfit = float(np.abs(dsamp).max()) * 1.3
    Rfit = min(max(np.ceil(Rfit * 50.0) / 50.0, 0.10), 0.90)

    return a, k_eff, vt8, tt8, wln, A, B, maxB, wln16, Rfit


last_run_info = {}


def kernel(v_hyp, t_hyp, c, _trace=False):
    c_val = float(np.asarray(c))
    a, k_eff, vt8, tt8, wln, A, B, maxB, wln16, Rfit = _host_prep(
        v_hyp, t_hyp, c_val
    )

    key = (c_val, Rfit)
    if key not in _program_cache:
        _program_cache[key] = _build_program(c_val, Rfit)
    nc = _program_cache[key]
    c0 = _cubic_fit(Rfit)[0]

    SA = np.array([A[k * R : (k + 1) * R].max() for k in range(NCORES)])
    in_maps = []
    for k in range(NCORES):
        rows = slice(k * R, (k + 1) * R)
        # bias[p, m] = (A_i - SA) + k*c0 for row i = k*R + m*128 + p
        bias_mat = (
            (A[rows] - SA[k] + k_eff * c0).reshape(MT, 128).T.astype(np.float32)
        )
        in_maps.append(
            {
                "vt8": np.ascontiguousarray(vt8[:, :, rows]),
                "tt8": tt8,
                "wln": wln,
                "bias": np.ascontiguousarray(bias_mat),
            }
        )

    def _aggregate_rowsums(rp):
        # [128, 44]: 32 (m, nq) slots + 8 half-chunk slots for (nq0, m<4)
        # + 4 quarter-chunk slots for the (nq3, m7) finale; the unused
        # normal slots are zeroed on device.
        rp_pm = rp[:, : MT * NQ].reshape(128, MT, NQ).sum(axis=2)  # [p, m]
        for m in range(4):
            rp_pm[:, m] += rp[:, 32 + 2 * m] + rp[:, 33 + 2 * m]
        rp_pm[:, MT - 1] += rp[:, 40:44].sum(axis=1)
        return rp_pm

    # Rare first-execution flake has been observed to return garbage once;
    # outputs are cheap to validate (row sums must be finite and positive),
    # so retry a couple of times if that happens.
    for attempt in range(3):
        res = run_bass_kernel_spmd(nc, in_maps, list(range(NCORES)), trace=_trace)
        last_run_info["results"] = res
        results = res.results
        ok = all(
            np.all(np.isfinite(results[k]["rowparts"]))
            and np.all(
                _aggregate_rowsums(results[k]["rowparts"].astype(np.float64)) > 0
            )
            and np.all(np.isfinite(results[k]["colsum"].astype(np.float32)))
            for k in range(NCORES)
        )
        if ok:
            break

    # device row sums are sum_j exp(x_ij - SA_k - maxB)
    rowLSE = np.empty(N, np.float64)
    colsum_parts = np.empty((NCORES, N), np.float64)
    for k in range(NCORES):
        rp_pm = _aggregate_rowsums(results[k]["rowparts"].astype(np.float64))
        rows = slice(k * R, (k + 1) * R)
        rowLSE[rows] = np.log(rp_pm.T.reshape(R)) + (SA[k] + maxB)
        colsum_parts[k] = results[k]["colsum"].astype(np.float64).sum(axis=0)

    loss_v2t = np.mean(rowLSE - a)
    M0 = SA.max()
    # wln16 rides inside the device exponent, so col sums are already
    # complete shifted-exp column sums
    col = (colsum_parts * np.exp(SA - M0)[:, None]).sum(axis=0)
    colLSE = np.log(col) + M0 + maxB
    loss_t2v = np.mean(colLSE - a)
    return np.asarray(0.5 * (loss_v2t + loss_t2v), dtype=np.float32)
